# revision 17
# baseline (speedup 1.0000x reference)
"""Cross-Covariance Attention (XCA) Bass/Tile kernel for Trainium2.

Problem: B=8, N=4096, DIM=768, H=8, DH=96 (fp32 in/out).
Sharding: data-parallel over batch -- core b handles batch b (8 cores).

Per-core dataflow (all big GEMMs in bf16, fp32 PSUM accumulation):
  host: xT = x[b].T cast to bf16  [768, 4096]
  1. q/k proj (token-major): q|k [128n, 1536] = xT_chunk.T @ Wqk_T  (+bias)
     v proj (channel-major): v [128c, 512n] = Wv_T.T @ xT           (+bias)
     squares -> per-channel norm^2 via ones-matmul (PSUM column accum)
     raw qk_h [96,96] += q_chunk_h.T @ k_chunk_h  (PSUM accum over chunks)
  2. rq = temp/sqrt(nq), rk = 1/sqrt(nk); scale qk via two PE transposes
     (per-partition tensor_scalar each side); softmax rows (ACT Exp fused sum)
  3. qkv_h [96c, 512n] = attnT_h.T @ v_h; repack to 6x[128hc, 4096] bf16
  4. out [128n, 768] = qkv_chunk.T @ Wo_T + bo -> DRAM (fp32)
"""

from contextlib import ExitStack

import numpy as np
import ml_dtypes

import concourse.bass as bass
import concourse.tile as tile
import concourse.mybir as mybir
from concourse import bacc
from concourse.bass_utils import run_bass_kernel_spmd

BF = mybir.dt.bfloat16
F8 = mybir.dt.float8e4
F32 = mybir.dt.float32
AX = mybir.AxisListType
OP = mybir.AluOpType
ACTF = mybir.ActivationFunctionType

D = 768
H = 8
DH = 96
N_CORES = 8
B = 8
N_TOK_FULL = 4096

# tunables (cost-model A/B'd)
XT_BUFS = 12
PROJ_BUFS = 3
QKSB_BUFS = 3
VTMP_BUFS = 4
QKVTMP_BUFS = 4
OUTSB_BUFS = 3

bf16 = ml_dtypes.bfloat16
f8e4 = ml_dtypes.float8_e4m3
W8_SCALE = 16.0  # fp8 weight pre-scale (undone in the bias drain)


def _head_spans(c_lo, c_hi):
    """Split channel range [c_lo, c_hi) of a 768-channel axis at head (96)
    boundaries. Yields (h, head_off, abs_off, length)."""
    c = c_lo
    while c < c_hi:
        h = c // DH
        end = min(c_hi, (h + 1) * DH)
        yield h, c - h * DH, c, end - c
        c = end


def _tile_spans(c_lo, c_hi):
    """Split channel range [c_lo, c_hi) at 128 (tile) boundaries.
    Yields (j, tile_off, abs_off, length)."""
    c = c_lo
    while c < c_hi:
        j = c // 128
        end = min(c_hi, (j + 1) * 128)
        yield j, c - j * 128, c, end - c
        c = end


def _emit(tc, n_tok, io, pfx=""):
    nc = tc.nc
    n_chunks = n_tok // 128
    n_sc = n_tok // 512
    SC = 512

    with ExitStack() as top:
        consts = top.enter_context(tc.tile_pool(name=pfx + "consts", bufs=1))
        vhead_pool = top.enter_context(tc.tile_pool(name=pfx + "vhead", bufs=1))

        # ---- constants (DMA order = need order: first q/k weight slice only;
        # the rest is emitted inside the sc=0 iteration after xT sc0) ----
        w_sb = [consts.tile([128, 2304], BF, name=f"wall{j}", tag=f"wall{j}")
                for j in range(6)]
        for j in range(6):
            nc.sync.dma_start(w_sb[j][:, 0:512],
                              io["w_all"][128 * j:128 * (j + 1), 0:512])
        bqk_sb = consts.tile([128, 1536], F32, name="bqk", tag="bqk")
        bv_sb = consts.tile([128, 6], F32, name="bv", tag="bv")
        wo_sb = [consts.tile([128, 768], BF, name=f"wo{j}", tag=f"wo{j}")
                 for j in range(6)]
        bo_sb = consts.tile([128, 768], F32, name="bo", tag="bo")
        tmp_sb = consts.tile([96, 8], F32, name="tempbc", tag="tempbc")
        nc.sync.dma_start(tmp_sb[:], io["tempbc"][:])
        idf = consts.tile([128, 128], F32, name="idf", tag="idf")
        nc.sync.dma_start(idf[:], io["idf"][:])
        idb = consts.tile([128, 128], BF, name="idb", tag="idb")
        nc.sync.dma_start(idb[:], io["idb"][:])
        ones_sb = consts.tile([128, 1], F32, name="ones", tag="ones")
        nc.sync.dma_start(ones_sb[:], io["ones"][:])

        v_head = []
        for h in range(H):
            t = vhead_pool.tile([96, n_tok], BF, name=f"vh{h}", tag=f"vh{h}")
            v_head.append(t)

        qsq_acc = consts.tile([128, 1536], F32, name="qsqacc", tag="qsqacc")

        mid = top.enter_context(ExitStack())  # spans phase 1 + phase 2a
        accps = mid.enter_context(
            tc.tile_pool(name=pfx + "accps", bufs=1, space=bass.MemorySpace.PSUM))
        # persistent PSUM accumulators (phase 1 -> phase 2a)
        qk_acc = accps.tile([96, 1024], F32, name="qkacc", tag="qkacc")
        norms_ps = accps.tile([96, 16], F32, name="normsps", tag="normsps")

        # ---- phase 1 ----
        with ExitStack() as ph1:
            xt_pool = ph1.enter_context(tc.tile_pool(name=pfx + "xt", bufs=XT_BUFS))
            projps = ph1.enter_context(
                tc.tile_pool(name=pfx + "projps", bufs=PROJ_BUFS, space=bass.MemorySpace.PSUM))
            qksb_pool = ph1.enter_context(tc.tile_pool(name=pfx + "qksbp", bufs=QKSB_BUFS))
            qksq_pool = ph1.enter_context(tc.tile_pool(name=pfx + "qksqp", bufs=2))
            vtmp_pool = ph1.enter_context(tc.tile_pool(name=pfx + "vtmpp", bufs=VTMP_BUFS))

            pending_qk = None

            def _emit_qk(pq):
                pi, pqksb = pq
                for h in range(H):
                    nc.tensor.matmul(
                        qk_acc[:, 128 * h:128 * h + 96],
                        pqksb[:, 96 * h:96 * (h + 1)],
                        pqksb[:, 768 + 96 * h:768 + 96 * (h + 1)],
                        start=(pi == 0 and h % 4 == 0),
                        stop=(pi == n_chunks - 1 and h % 4 == 3))

            for sc in range(n_sc):
                xts = []
                for j in range(6):
                    t = xt_pool.tile([128, SC], BF, name=f"xt{j}_{sc}", tag="xt")
                    nc.sync.dma_start(
                        t[:], io["xT"][128 * j:128 * (j + 1), SC * sc:SC * (sc + 1)])
                    xts.append(t)
                if sc == 0:
                    # remaining constants, in need order, behind xT sc0
                    for j in range(6):
                        nc.sync.dma_start(w_sb[j][:, 512:1536],
                                          io["w_all"][128 * j:128 * (j + 1), 512:1536])
                    nc.sync.dma_start(bqk_sb[:], io["bqk"][:])
                    for j in range(6):
                        nc.sync.dma_start(w_sb[j][:, 1536:2304],
                                          io["w_all"][128 * j:128 * (j + 1), 1536:2304])
                    nc.sync.dma_start(bv_sb[:], io["bv"][:])

                # q/k projections per 128-token chunk; qk matmuls pipelined
                # one chunk behind so PE never waits on the bias drains.
                for il in range(4):
                    i = 4 * sc + il
                    qksb = qksb_pool.tile([128, 1536], BF, name=f"qksb{i}", tag="qksb")
                    for g in range(3):
                        pps = projps.tile([128, 512], F32, name=f"pps{g}_{i}", tag="pp")
                        for j in range(6):
                            nc.tensor.matmul(
                                pps[:],
                                xts[j][:, 128 * il:128 * (il + 1)],
                                w_sb[j][:, 512 * g:512 * (g + 1)],
                                start=(j == 0), stop=(j == 5))
                        nc.vector.tensor_add(
                            qksb[:, 512 * g:512 * (g + 1)], pps[:],
                            bqk_sb[:, 512 * g:512 * (g + 1)])
                    qsq = qksq_pool.tile([128, 1536], BF, name=f"qsq{i}", tag="qksq")
                    nc.scalar.square(qsq[:], qksb[:])
                    if i == 0:
                        nc.gpsimd.tensor_copy(qsq_acc[:], qsq[:])
                    else:
                        nc.gpsimd.tensor_tensor(
                            qsq_acc[:], qsq_acc[:], qsq[:], op=OP.add)
                    if pending_qk is not None:
                        _emit_qk(pending_qk)
                    pending_qk = (i, qksb)

                # v projection (channel-major) for this superchunk
                for m in range(6):
                    vps = projps.tile([128, SC], F32, name=f"vps{m}_{sc}", tag="pp")
                    for j in range(6):
                        nc.tensor.matmul(
                            vps[:],
                            w_sb[j][:, 1536 + 128 * m:1536 + 128 * (m + 1)],
                            xts[j][:],
                            start=(j == 0), stop=(j == 5))
                    vtmp = vtmp_pool.tile([128, SC], BF, name=f"vt{m}_{sc}", tag="vtmp")
                    nc.vector.tensor_scalar_add(vtmp[:], vps[:], bv_sb[:, m:m + 1])
                    # repack to head-aligned tiles (partition-shift => DMA)
                    for h, hoff, cabs, ln in _head_spans(128 * m, 128 * (m + 1)):
                        src0 = cabs - 128 * m
                        nc.sync.dma_start(
                            v_head[h][hoff:hoff + ln, SC * sc:SC * (sc + 1)],
                            vtmp[src0:src0 + ln, :])

            _emit_qk(pending_qk)

        # ---- phase 2a: norms -> scales -> softmax -> attnT ----
        attnT_sb = []
        with ExitStack() as ph2a:
            smalls = ph2a.enter_context(tc.tile_pool(name=pfx + "smalls", bufs=1))
            tps = ph2a.enter_context(
                tc.tile_pool(name=pfx + "tps", bufs=2, space=bass.MemorySpace.PSUM))

            for s in range(16):
                nc.tensor.matmul(
                    norms_ps[:, s:s + 1],
                    qsq_acc[:, 96 * s:96 * (s + 1)],
                    ones_sb[:],
                    start=True, stop=True)
            sn_sb = smalls.tile([96, 16], F32, name="snsb", tag="snsb")
            nc.scalar.sqrt(sn_sb[:], norms_ps[:])
            rn_sb = smalls.tile([96, 16], F32, name="rnsb", tag="rnsb")
            nc.vector.reciprocal(rn_sb[:], sn_sb[:])
            rq_sb = smalls.tile([96, 8], F32, name="rqsb", tag="rqsb")
            nc.vector.tensor_mul(rq_sb[:], rn_sb[:, 0:8], tmp_sb[:])

            qkraw = smalls.tile([96, 768], F32, name="qkraw", tag="qkraw")
            for h in range(H):
                nc.vector.tensor_copy(
                    qkraw[:, 96 * h:96 * (h + 1)], qk_acc[:, 128 * h:128 * h + 96])

            for h in range(H):
                t1 = tps.tile([96, 96], F32, name=f"t1_{h}", tag="tp")
                nc.tensor.transpose(t1[:], qkraw[:, 96 * h:96 * (h + 1)],
                                    idf[0:96, 0:96])
                t1s = smalls.tile([96, 96], F32, name=f"t1s{h}", tag="t1s", bufs=2)
                nc.vector.tensor_scalar_mul(t1s[:], t1[:], rn_sb[:, 8 + h:9 + h])
                t2 = tps.tile([96, 96], F32, name=f"t2_{h}", tag="tp")
                nc.tensor.transpose(t2[:], t1s[:], idf[0:96, 0:96])
                smin = smalls.tile([96, 96], F32, name=f"smin{h}", tag="smin", bufs=2)
                nc.vector.tensor_scalar_mul(smin[:], t2[:], rq_sb[:, h:h + 1])
                negmax = smalls.tile([96, 1], F32, name=f"ngm{h}", tag="ngm", bufs=2)
                nc.vector.tensor_reduce(
                    negmax[:], smin[:], axis=AX.X, op=OP.max, negate=True)
                esb = smalls.tile([96, 96], F32, name=f"esb{h}", tag="esb", bufs=2)
                esum = smalls.tile([96, 1], F32, name=f"esum{h}", tag="esum", bufs=2)
                nc.scalar.activation(
                    esb[:], smin[:], ACTF.Exp,
                    bias=negmax[:, 0:1], scale=1.0, accum_out=esum[:])
                rsum = smalls.tile([96, 1], F32, name=f"rsum{h}", tag="rsum", bufs=2)
                nc.vector.reciprocal(rsum[:], esum[:])
                attn = smalls.tile([96, 96], BF, name=f"attn{h}", tag="attn", bufs=2)
                nc.vector.tensor_scalar_mul(attn[:], esb[:], rsum[:, 0:1])
                t3 = tps.tile([96, 96], BF, name=f"t3_{h}", tag="tp3")
                nc.tensor.transpose(t3[:], attn[:], idb[0:96, 0:96])
                at = consts.tile([96, 96], BF, name=f"attnT{h}", tag=f"attnT{h}")
                nc.vector.tensor_copy(at[:], t3[:])
                attnT_sb.append(at)

        mid.close()  # release qk_acc / norms_ps PSUM banks

        # ---- phase 2b: qkv + repack; phase 3: out projection ----
        with ExitStack() as ph3:
            qkvps = ph3.enter_context(
                tc.tile_pool(name=pfx + "qkvps", bufs=3, space=bass.MemorySpace.PSUM))
            qkvtmp_pool = ph3.enter_context(tc.tile_pool(name=pfx + "qkvtmp", bufs=QKVTMP_BUFS))
            qkvcm_pool = ph3.enter_context(tc.tile_pool(name=pfx + "qkvcm", bufs=1))
            outps = ph3.enter_context(
                tc.tile_pool(name=pfx + "outps", bufs=2, space=bass.MemorySpace.PSUM))
            outsb_pool = ph3.enter_context(tc.tile_pool(name=pfx + "outsb", bufs=OUTSB_BUFS))

            # wo/bo arrive now (deferred so startup DMAs serve phase 1 first)
            for j in range(6):
                nc.sync.dma_start(wo_sb[j][:], io["wo"][128 * j:128 * (j + 1), :])
            nc.sync.dma_start(bo_sb[:], io["bo"][:])

            qkv_cm = []
            for j in range(6):
                t = qkvcm_pool.tile([128, n_tok], BF, name=f"qc{j}", tag=f"qc{j}")
                qkv_cm.append(t)

            def _emit_out(sc):
                for i in range(4 * sc, 4 * (sc + 1)):
                    opst = outps.tile([128, 768], F32, name=f"ops{i}", tag="outps")
                    for nf0, nfw in ((0, 512), (512, 256)):
                        for j in range(6):
                            nc.tensor.matmul(
                                opst[:, nf0:nf0 + nfw],
                                qkv_cm[j][:, 128 * i:128 * (i + 1)],
                                wo_sb[j][:, nf0:nf0 + nfw],
                                start=(j == 0), stop=(j == 5))
                    osb = outsb_pool.tile([128, 768], F32, name=f"osb{i}", tag="outsb")
                    nc.vector.tensor_add(osb[:], opst[:], bo_sb[:])
                    nc.sync.dma_start(io["out"][128 * i:128 * (i + 1), :], osb[:])

            for sc in range(n_sc):
                for h in range(H):
                    qp = qkvps.tile([96, SC], F32, name=f"qp{h}_{sc}", tag="qkvps")
                    nc.tensor.matmul(qp[:], attnT_sb[h][:],
                                     v_head[h][:, SC * sc:SC * (sc + 1)])
                    qt = qkvtmp_pool.tile([96, SC], BF, name=f"qt{h}_{sc}", tag="qkvtmp")
                    nc.vector.tensor_copy(qt[:], qp[:])
                    for j, joff, cabs, ln in _tile_spans(96 * h, 96 * (h + 1)):
                        src0 = cabs - 96 * h
                        nc.sync.dma_start(
                            qkv_cm[j][joff:joff + ln, SC * sc:SC * (sc + 1)],
                            qt[src0:src0 + ln, :])
                if sc >= 1:
                    _emit_out(sc - 1)
            _emit_out(n_sc - 1)


def build_nc(n_tok=N_TOK_FULL, repeat=1):
    nc = bacc.Bacc("TRN2", target_bir_lowering=False, debug=False)
    io = {
        "xT": nc.dram_tensor("xT", [D, n_tok], BF, kind="ExternalInput").ap(),
        "w_all": nc.dram_tensor("w_all", [D, 2304], BF, kind="ExternalInput").ap(),
        "wo": nc.dram_tensor("wo", [D, D], BF, kind="ExternalInput").ap(),
        "bqk": nc.dram_tensor("bqk", [128, 1536], F32, kind="ExternalInput").ap(),
        "bv": nc.dram_tensor("bv", [128, 6], F32, kind="ExternalInput").ap(),
        "bo": nc.dram_tensor("bo", [128, 768], F32, kind="ExternalInput").ap(),
        "tempbc": nc.dram_tensor("tempbc", [96, 8], F32, kind="ExternalInput").ap(),
        "idf": nc.dram_tensor("idf", [128, 128], F32, kind="ExternalInput").ap(),
        "idb": nc.dram_tensor("idb", [128, 128], BF, kind="ExternalInput").ap(),
        "ones": nc.dram_tensor("ones", [128, 1], F32, kind="ExternalInput").ap(),
        "out": nc.dram_tensor("out", [n_tok, D], F32, kind="ExternalOutput").ap(),
    }
    with tile.TileContext(nc) as tc:
        for r in range(repeat):
            _emit(tc, n_tok, io, pfx=f"r{r}_" if repeat > 1 else "")
    nc.compile()
    return nc


def host_prep(x, Wq, bq, Wk, bk, Wv, bv, temp, Wo, bo, n_tok=N_TOK_FULL):
    """Build per-core input maps from the full-problem inputs."""
    x = np.asarray(x, dtype=np.float32)
    Wq_f = np.asarray(Wq, dtype=np.float32).reshape(H * DH, D)
    Wk_f = np.asarray(Wk, dtype=np.float32).reshape(H * DH, D)
    Wv_f = np.asarray(Wv, dtype=np.float32).reshape(H * DH, D)
    w_all = np.ascontiguousarray(
        np.concatenate([Wq_f, Wk_f, Wv_f], axis=0).T).astype(bf16)  # [768, 2304]
    wo_t = np.ascontiguousarray(np.asarray(Wo, dtype=np.float32).T).astype(bf16)
    bqk_row = np.concatenate(
        [np.asarray(bq, np.float32).reshape(-1), np.asarray(bk, np.float32).reshape(-1)])
    bqk_bc = np.ascontiguousarray(np.broadcast_to(bqk_row[None, :], (128, 1536)),
                                  dtype=np.float32)
    bv_col = np.ascontiguousarray(
        np.asarray(bv, np.float32).reshape(-1).reshape(6, 128).T)  # [128, 6]
    bo_bc = np.ascontiguousarray(
        np.broadcast_to(np.asarray(bo, np.float32)[None, :], (128, 768)),
        dtype=np.float32)
    temp_bc = np.ascontiguousarray(
        np.broadcast_to(np.asarray(temp, np.float32)[None, :], (96, H)),
        dtype=np.float32)
    idf = np.eye(128, dtype=np.float32)
    idb = np.eye(128, dtype=np.float32).astype(bf16)
    ones = np.ones((128, 1), dtype=np.float32)

    shared = {
        "w_all": w_all, "wo": wo_t, "bqk": bqk_bc, "bv": bv_col, "bo": bo_bc,
        "tempbc": temp_bc, "idf": idf, "idb": idb, "ones": ones,
    }
    in_maps = []
    for b in range(x.shape[0]):
        xT = np.ascontiguousarray(x[b, :n_tok, :].T).astype(bf16)  # [768, n_tok]
        in_maps.append({"xT": xT, **shared})
    return in_maps


_NC_CACHE = {}


def kernel(**inputs):
    n_tok = N_TOK_FULL
    if "nc" not in _NC_CACHE:
        _NC_CACHE["nc"] = build_nc(n_tok)
    nc = _NC_CACHE["nc"]
    in_maps = host_prep(**inputs, n_tok=n_tok)
    res = run_bass_kernel_spmd(nc, in_maps, core_ids=list(range(N_CORES)))
    out = np.stack([res.results[c]["out"] for c in range(N_CORES)], axis=0)
    return out.astype(np.float32)


if __name__ == "__main__":
    import reference
    inputs = reference.setup_inputs()
    out = kernel(**{k: np.asarray(v) for k, v in inputs.items()})
    print("out", out.shape, out.dtype)


# revision 18
# speedup vs baseline: 3.4592x; 3.4592x over previous
"""Cross-Covariance Attention (XCA) Bass/Tile kernel for Trainium2.

Problem: B=8, N=4096, DIM=768, H=8, DH=96 (fp32 in/out).
Sharding: data-parallel over batch -- core b handles batch b (8 cores).

Per-core dataflow (all big GEMMs in bf16, fp32 PSUM accumulation):
  host: xT = x[b].T cast to bf16  [768, 4096]
  1. q/k proj (token-major): q|k [128n, 1536] = xT_chunk.T @ Wqk_T  (+bias)
     v proj (channel-major): v [128c, 512n] = Wv_T.T @ xT           (+bias)
     squares -> per-channel norm^2 via ones-matmul (PSUM column accum)
     raw qk_h [96,96] += q_chunk_h.T @ k_chunk_h  (PSUM accum over chunks)
  2. rq = temp/sqrt(nq), rk = 1/sqrt(nk); scale qk via two PE transposes
     (per-partition tensor_scalar each side); softmax rows (ACT Exp fused sum)
  3. qkv_h [96c, 512n] = attnT_h.T @ v_h; repack to 6x[128hc, 4096] bf16
  4. out [128n, 768] = qkv_chunk.T @ Wo_T + bo -> DRAM (fp32)
"""

from contextlib import ExitStack

import numpy as np
import ml_dtypes

import concourse.bass as bass
import concourse.tile as tile
import concourse.mybir as mybir
from concourse import bacc
from concourse.bass_utils import run_bass_kernel_spmd

BF = mybir.dt.bfloat16
F8 = mybir.dt.float8e4
F32 = mybir.dt.float32
AX = mybir.AxisListType
OP = mybir.AluOpType
ACTF = mybir.ActivationFunctionType

D = 768
H = 8
DH = 96
N_CORES = 8
B = 8
N_TOK_FULL = 4096

# tunables (cost-model A/B'd)
XT_BUFS = 18
PROJ_BUFS = 4
QKSB_BUFS = 4
VTMP_BUFS = 6
QKVTMP_BUFS = 6
OUTSB_BUFS = 4

bf16 = ml_dtypes.bfloat16
f8e4 = ml_dtypes.float8_e4m3
W8_SCALE = 16.0  # fp8 weight pre-scale (undone in the bias drain)


def _head_spans(c_lo, c_hi):
    """Split channel range [c_lo, c_hi) of a 768-channel axis at head (96)
    boundaries. Yields (h, head_off, abs_off, length)."""
    c = c_lo
    while c < c_hi:
        h = c // DH
        end = min(c_hi, (h + 1) * DH)
        yield h, c - h * DH, c, end - c
        c = end


def _tile_spans(c_lo, c_hi):
    """Split channel range [c_lo, c_hi) at 128 (tile) boundaries.
    Yields (j, tile_off, abs_off, length)."""
    c = c_lo
    while c < c_hi:
        j = c // 128
        end = min(c_hi, (j + 1) * 128)
        yield j, c - j * 128, c, end - c
        c = end


def _emit(tc, n_tok, io, pfx=""):
    nc = tc.nc
    n_chunks = n_tok // 128
    n_sc = n_tok // 512
    SC = 512

    with ExitStack() as top:
        consts = top.enter_context(tc.tile_pool(name=pfx + "consts", bufs=1))
        vhead_pool = top.enter_context(tc.tile_pool(name=pfx + "vhead", bufs=1))

        # ---- constants (DMA order = need order: first q/k weight slice only;
        # the rest is emitted inside the sc=0 iteration after xT sc0) ----
        w_sb = [consts.tile([128, 2304], BF, name=f"wall{j}", tag=f"wall{j}")
                for j in range(6)]
        for j in range(6):
            nc.sync.dma_start(w_sb[j][:, 0:512],
                              io["w_all"][128 * j:128 * (j + 1), 0:512])
        bqk_sb = consts.tile([128, 1536], F32, name="bqk", tag="bqk")
        bv_sb = consts.tile([128, 6], F32, name="bv", tag="bv")
        wo_sb = [consts.tile([128, 768], BF, name=f"wo{j}", tag=f"wo{j}")
                 for j in range(6)]
        bo_sb = consts.tile([128, 768], F32, name="bo", tag="bo")
        tmp_sb = consts.tile([96, 8], F32, name="tempbc", tag="tempbc")
        nc.sync.dma_start(tmp_sb[:], io["tempbc"][:])
        idf = consts.tile([128, 128], F32, name="idf", tag="idf")
        nc.sync.dma_start(idf[:], io["idf"][:])
        idb = consts.tile([128, 128], BF, name="idb", tag="idb")
        nc.sync.dma_start(idb[:], io["idb"][:])
        ones_sb = consts.tile([128, 1], F32, name="ones", tag="ones")
        nc.sync.dma_start(ones_sb[:], io["ones"][:])

        v_head = []
        for h in range(H):
            t = vhead_pool.tile([96, n_tok], BF, name=f"vh{h}", tag=f"vh{h}")
            v_head.append(t)

        qsq_acc = consts.tile([128, 1536], F32, name="qsqacc", tag="qsqacc")

        mid = top.enter_context(ExitStack())  # spans phase 1 + phase 2a
        accps = mid.enter_context(
            tc.tile_pool(name=pfx + "accps", bufs=1, space=bass.MemorySpace.PSUM))
        # persistent PSUM accumulators (phase 1 -> phase 2a)
        qk_acc = accps.tile([96, 1024], F32, name="qkacc", tag="qkacc")
        norms_ps = accps.tile([96, 16], F32, name="normsps", tag="normsps")

        # ---- phase 1 ----
        with ExitStack() as ph1:
            xt_pool = ph1.enter_context(tc.tile_pool(name=pfx + "xt", bufs=XT_BUFS))
            projps = ph1.enter_context(
                tc.tile_pool(name=pfx + "projps", bufs=PROJ_BUFS, space=bass.MemorySpace.PSUM))
            qksb_pool = ph1.enter_context(tc.tile_pool(name=pfx + "qksbp", bufs=QKSB_BUFS))
            qksq_pool = ph1.enter_context(tc.tile_pool(name=pfx + "qksqp", bufs=2))
            vtmp_pool = ph1.enter_context(tc.tile_pool(name=pfx + "vtmpp", bufs=VTMP_BUFS))

            pending_qk = None

            def _emit_qk(pq):
                pi, pqksb = pq
                for h in range(H):
                    nc.tensor.matmul(
                        qk_acc[:, 128 * h:128 * h + 96],
                        pqksb[:, 96 * h:96 * (h + 1)],
                        pqksb[:, 768 + 96 * h:768 + 96 * (h + 1)],
                        start=(pi == 0 and h % 4 == 0),
                        stop=(pi == n_chunks - 1 and h % 4 == 3))

            for sc in range(n_sc):
                xts = []
                for j in range(6):
                    t = xt_pool.tile([128, SC], BF, name=f"xt{j}_{sc}", tag="xt")
                    nc.sync.dma_start(
                        t[:], io["xT"][128 * j:128 * (j + 1), SC * sc:SC * (sc + 1)])
                    xts.append(t)
                if sc == 0:
                    # remaining constants, in need order, behind xT sc0
                    for j in range(6):
                        nc.sync.dma_start(w_sb[j][:, 512:1536],
                                          io["w_all"][128 * j:128 * (j + 1), 512:1536])
                    nc.sync.dma_start(bqk_sb[:], io["bqk"][:])
                    for j in range(6):
                        nc.sync.dma_start(w_sb[j][:, 1536:2304],
                                          io["w_all"][128 * j:128 * (j + 1), 1536:2304])
                    nc.sync.dma_start(bv_sb[:], io["bv"][:])

                # q/k projections per 128-token chunk; qk matmuls pipelined
                # one chunk behind so PE never waits on the bias drains.
                for il in range(4):
                    i = 4 * sc + il
                    qksb = qksb_pool.tile([128, 1536], BF, name=f"qksb{i}", tag="qksb")
                    for g in range(3):
                        pps = projps.tile([128, 512], F32, name=f"pps{g}_{i}", tag="pp")
                        for j in range(6):
                            nc.tensor.matmul(
                                pps[:],
                                xts[j][:, 128 * il:128 * (il + 1)],
                                w_sb[j][:, 512 * g:512 * (g + 1)],
                                start=(j == 0), stop=(j == 5))
                        nc.vector.tensor_add(
                            qksb[:, 512 * g:512 * (g + 1)], pps[:],
                            bqk_sb[:, 512 * g:512 * (g + 1)])
                    qsq = qksq_pool.tile([128, 1536], BF, name=f"qsq{i}", tag="qksq")
                    nc.scalar.square(qsq[:], qksb[:])
                    if i == 0:
                        nc.gpsimd.tensor_copy(qsq_acc[:], qsq[:])
                    else:
                        nc.gpsimd.tensor_tensor(
                            qsq_acc[:], qsq_acc[:], qsq[:], op=OP.add)
                    if pending_qk is not None:
                        _emit_qk(pending_qk)
                    pending_qk = (i, qksb)

                # v projection (channel-major) for this superchunk
                for m in range(6):
                    vps = projps.tile([128, SC], F32, name=f"vps{m}_{sc}", tag="pp")
                    for j in range(6):
                        nc.tensor.matmul(
                            vps[:],
                            w_sb[j][:, 1536 + 128 * m:1536 + 128 * (m + 1)],
                            xts[j][:],
                            start=(j == 0), stop=(j == 5))
                    vtmp = vtmp_pool.tile([128, SC], BF, name=f"vt{m}_{sc}", tag="vtmp")
                    nc.scalar.activation(vtmp[:], vps[:], ACTF.Identity,
                                         bias=bv_sb[:, m:m + 1], scale=1.0)
                    # repack to head-aligned tiles (partition-shift => DMA)
                    for h, hoff, cabs, ln in _head_spans(128 * m, 128 * (m + 1)):
                        src0 = cabs - 128 * m
                        nc.sync.dma_start(
                            v_head[h][hoff:hoff + ln, SC * sc:SC * (sc + 1)],
                            vtmp[src0:src0 + ln, :])

            _emit_qk(pending_qk)

        # ---- phase 2a: norms -> scales -> softmax -> attnT ----
        attnT_sb = []
        with ExitStack() as ph2a:
            smalls = ph2a.enter_context(tc.tile_pool(name=pfx + "smalls", bufs=1))
            tps = ph2a.enter_context(
                tc.tile_pool(name=pfx + "tps", bufs=2, space=bass.MemorySpace.PSUM))

            for s in range(16):
                nc.tensor.matmul(
                    norms_ps[:, s:s + 1],
                    qsq_acc[:, 96 * s:96 * (s + 1)],
                    ones_sb[:],
                    start=True, stop=True)
            sn_sb = smalls.tile([96, 16], F32, name="snsb", tag="snsb")
            nc.scalar.sqrt(sn_sb[:], norms_ps[:])
            rn_sb = smalls.tile([96, 16], F32, name="rnsb", tag="rnsb")
            nc.vector.reciprocal(rn_sb[:], sn_sb[:])
            rq_sb = smalls.tile([96, 8], F32, name="rqsb", tag="rqsb")
            nc.vector.tensor_mul(rq_sb[:], rn_sb[:, 0:8], tmp_sb[:])

            qkraw = smalls.tile([96, 768], F32, name="qkraw", tag="qkraw")
            for h in range(H):
                nc.scalar.copy(
                    qkraw[:, 96 * h:96 * (h + 1)], qk_acc[:, 128 * h:128 * h + 96])

            for h in range(H):
                t1 = tps.tile([96, 96], F32, name=f"t1_{h}", tag="tp")
                nc.tensor.transpose(t1[:], qkraw[:, 96 * h:96 * (h + 1)],
                                    idf[0:96, 0:96])
                t1s = smalls.tile([96, 96], F32, name=f"t1s{h}", tag="t1s", bufs=2)
                nc.vector.tensor_scalar_mul(t1s[:], t1[:], rn_sb[:, 8 + h:9 + h])
                t2 = tps.tile([96, 96], F32, name=f"t2_{h}", tag="tp")
                nc.tensor.transpose(t2[:], t1s[:], idf[0:96, 0:96])
                smin = smalls.tile([96, 96], F32, name=f"smin{h}", tag="smin", bufs=2)
                nc.vector.tensor_scalar_mul(smin[:], t2[:], rq_sb[:, h:h + 1])
                negmax = smalls.tile([96, 1], F32, name=f"ngm{h}", tag="ngm", bufs=2)
                nc.vector.tensor_reduce(
                    negmax[:], smin[:], axis=AX.X, op=OP.max, negate=True)
                esb = smalls.tile([96, 96], F32, name=f"esb{h}", tag="esb", bufs=2)
                esum = smalls.tile([96, 1], F32, name=f"esum{h}", tag="esum", bufs=2)
                nc.scalar.activation(
                    esb[:], smin[:], ACTF.Exp,
                    bias=negmax[:, 0:1], scale=1.0, accum_out=esum[:])
                rsum = smalls.tile([96, 1], F32, name=f"rsum{h}", tag="rsum", bufs=2)
                nc.vector.reciprocal(rsum[:], esum[:])
                attn = smalls.tile([96, 96], BF, name=f"attn{h}", tag="attn", bufs=2)
                nc.vector.tensor_scalar_mul(attn[:], esb[:], rsum[:, 0:1])
                t3 = tps.tile([96, 96], BF, name=f"t3_{h}", tag="tp3")
                nc.tensor.transpose(t3[:], attn[:], idb[0:96, 0:96])
                at = consts.tile([96, 96], BF, name=f"attnT{h}", tag=f"attnT{h}")
                nc.scalar.copy(at[:], t3[:])
                attnT_sb.append(at)

        mid.close()  # release qk_acc / norms_ps PSUM banks

        # ---- phase 2b: qkv + repack; phase 3: out projection ----
        with ExitStack() as ph3:
            qkvps = ph3.enter_context(
                tc.tile_pool(name=pfx + "qkvps", bufs=3, space=bass.MemorySpace.PSUM))
            qkvtmp_pool = ph3.enter_context(tc.tile_pool(name=pfx + "qkvtmp", bufs=QKVTMP_BUFS))
            qkvcm_pool = ph3.enter_context(tc.tile_pool(name=pfx + "qkvcm", bufs=1))
            outps = ph3.enter_context(
                tc.tile_pool(name=pfx + "outps", bufs=2, space=bass.MemorySpace.PSUM))
            outsb_pool = ph3.enter_context(tc.tile_pool(name=pfx + "outsb", bufs=OUTSB_BUFS))

            # wo/bo arrive now (deferred so startup DMAs serve phase 1 first)
            for j in range(6):
                nc.sync.dma_start(wo_sb[j][:], io["wo"][128 * j:128 * (j + 1), :])
            nc.sync.dma_start(bo_sb[:], io["bo"][:])

            qkv_cm = []
            for j in range(6):
                t = qkvcm_pool.tile([128, n_tok], BF, name=f"qc{j}", tag=f"qc{j}")
                qkv_cm.append(t)

            def _emit_out(sc):
                for i in range(4 * sc, 4 * (sc + 1)):
                    opst = outps.tile([128, 768], F32, name=f"ops{i}", tag="outps")
                    for nf0, nfw in ((0, 512), (512, 256)):
                        for j in range(6):
                            nc.tensor.matmul(
                                opst[:, nf0:nf0 + nfw],
                                qkv_cm[j][:, 128 * i:128 * (i + 1)],
                                wo_sb[j][:, nf0:nf0 + nfw],
                                start=(j == 0), stop=(j == 5))
                    osb = outsb_pool.tile([128, 768], F32, name=f"osb{i}", tag="outsb")
                    nc.vector.tensor_add(osb[:], opst[:], bo_sb[:])
                    nc.sync.dma_start(io["out"][128 * i:128 * (i + 1), :], osb[:])

            for sc in range(n_sc):
                for h in range(H):
                    qp = qkvps.tile([96, SC], F32, name=f"qp{h}_{sc}", tag="qkvps")
                    nc.tensor.matmul(qp[:], attnT_sb[h][:],
                                     v_head[h][:, SC * sc:SC * (sc + 1)])
                    qt = qkvtmp_pool.tile([96, SC], BF, name=f"qt{h}_{sc}", tag="qkvtmp")
                    nc.vector.tensor_copy(qt[:], qp[:])
                    for j, joff, cabs, ln in _tile_spans(96 * h, 96 * (h + 1)):
                        src0 = cabs - 96 * h
                        nc.sync.dma_start(
                            qkv_cm[j][joff:joff + ln, SC * sc:SC * (sc + 1)],
                            qt[src0:src0 + ln, :])
                if sc >= 1:
                    _emit_out(sc - 1)
            _emit_out(n_sc - 1)


def build_nc(n_tok=N_TOK_FULL, repeat=1):
    nc = bacc.Bacc("TRN2", target_bir_lowering=False, debug=False)
    io = {
        "xT": nc.dram_tensor("xT", [D, n_tok], BF, kind="ExternalInput").ap(),
        "w_all": nc.dram_tensor("w_all", [D, 2304], BF, kind="ExternalInput").ap(),
        "wo": nc.dram_tensor("wo", [D, D], BF, kind="ExternalInput").ap(),
        "bqk": nc.dram_tensor("bqk", [128, 1536], F32, kind="ExternalInput").ap(),
        "bv": nc.dram_tensor("bv", [128, 6], F32, kind="ExternalInput").ap(),
        "bo": nc.dram_tensor("bo", [128, 768], F32, kind="ExternalInput").ap(),
        "tempbc": nc.dram_tensor("tempbc", [96, 8], F32, kind="ExternalInput").ap(),
        "idf": nc.dram_tensor("idf", [128, 128], F32, kind="ExternalInput").ap(),
        "idb": nc.dram_tensor("idb", [128, 128], BF, kind="ExternalInput").ap(),
        "ones": nc.dram_tensor("ones", [128, 1], F32, kind="ExternalInput").ap(),
        "out": nc.dram_tensor("out", [n_tok, D], F32, kind="ExternalOutput").ap(),
    }
    with tile.TileContext(nc) as tc:
        for r in range(repeat):
            _emit(tc, n_tok, io, pfx=f"r{r}_" if repeat > 1 else "")
    nc.compile()
    return nc


def host_prep(x, Wq, bq, Wk, bk, Wv, bv, temp, Wo, bo, n_tok=N_TOK_FULL):
    """Build per-core input maps from the full-problem inputs."""
    x = np.asarray(x, dtype=np.float32)
    Wq_f = np.asarray(Wq, dtype=np.float32).reshape(H * DH, D)
    Wk_f = np.asarray(Wk, dtype=np.float32).reshape(H * DH, D)
    Wv_f = np.asarray(Wv, dtype=np.float32).reshape(H * DH, D)
    w_all = np.ascontiguousarray(
        np.concatenate([Wq_f, Wk_f, Wv_f], axis=0).T).astype(bf16)  # [768, 2304]
    wo_t = np.ascontiguousarray(np.asarray(Wo, dtype=np.float32).T).astype(bf16)
    bqk_row = np.concatenate(
        [np.asarray(bq, np.float32).reshape(-1), np.asarray(bk, np.float32).reshape(-1)])
    bqk_bc = np.ascontiguousarray(np.broadcast_to(bqk_row[None, :], (128, 1536)),
                                  dtype=np.float32)
    bv_col = np.ascontiguousarray(
        np.asarray(bv, np.float32).reshape(-1).reshape(6, 128).T)  # [128, 6]
    bo_bc = np.ascontiguousarray(
        np.broadcast_to(np.asarray(bo, np.float32)[None, :], (128, 768)),
        dtype=np.float32)
    temp_bc = np.ascontiguousarray(
        np.broadcast_to(np.asarray(temp, np.float32)[None, :], (96, H)),
        dtype=np.float32)
    idf = np.eye(128, dtype=np.float32)
    idb = np.eye(128, dtype=np.float32).astype(bf16)
    ones = np.ones((128, 1), dtype=np.float32)

    shared = {
        "w_all": w_all, "wo": wo_t, "bqk": bqk_bc, "bv": bv_col, "bo": bo_bc,
        "tempbc": temp_bc, "idf": idf, "idb": idb, "ones": ones,
    }
    in_maps = []
    for b in range(x.shape[0]):
        xT = np.ascontiguousarray(x[b, :n_tok, :].T).astype(bf16)  # [768, n_tok]
        in_maps.append({"xT": xT, **shared})
    return in_maps


_NC_CACHE = {}


def kernel(**inputs):
    n_tok = N_TOK_FULL
    if "nc" not in _NC_CACHE:
        _NC_CACHE["nc"] = build_nc(n_tok)
    nc = _NC_CACHE["nc"]
    in_maps = host_prep(**inputs, n_tok=n_tok)
    res = run_bass_kernel_spmd(nc, in_maps, core_ids=list(range(N_CORES)))
    out = np.stack([res.results[c]["out"] for c in range(N_CORES)], axis=0)
    return out.astype(np.float32)


if __name__ == "__main__":
    import reference
    inputs = reference.setup_inputs()
    out = kernel(**{k: np.asarray(v) for k, v in inputs.items()})
    print("out", out.shape, out.dtype)
